# revision 11
# baseline (speedup 1.0000x reference)
"""Trainium2 Bass kernel for PersonalizedSimpleAttention.

Computation (per batch b, person p = person_idxs[b]):
    keys    = x @ (PK_W[p] @ Wk).T + PK_b[p]               # folded projection
    queries = x @ (PQ_W[p] @ Wq / sqrt(KH)).T + PQ_b[p] / sqrt(KH)
    v       = x @ Wv.T
    attn    = softmax(queries @ keys.T + maskbias, axis=-1)
    out     = attn @ v                                     # [T, VH]

The personalized [KH,KH] @ [KH,EMB] fold happens on-device (4 small matmuls
per batch) and removes the k0/q0 intermediates of the naive two-stage form.

Sharding: data-parallel over batch across 8 cores (8 batches each); the
per-person weight stacks are gathered on the host (pure indexing) so each
core receives exactly its 8 weight matrices.  All on-device layouts are
transposed ([feature, token]) so every matmul contracts over the partition
dim with no on-device transposes; softmax runs over the partition (key) dim
via a pairwise DVE/GPSIMD adder tree + gpsimd partition_all_reduce, and the
normalization is folded in after the attn@v matmul (divide by denom once on
[KH, T] instead of on [T, T]).

Matmul operand dtype is selectable (bf16 default; f32r = TF32-like; f32)
with fp32 PSUM accumulation throughout.
"""
import math
import os

import numpy as np

import concourse.bass as bass  # noqa: F401  (registers engines)
import concourse.mybir as mybir
from concourse import bacc
from concourse.bass_utils import run_bass_kernel_spmd
from concourse.tile import TileContext

F32 = mybir.dt.float32
AF = mybir.ActivationFunctionType

B, T, EMB, KH = 64, 1024, 128, 256
NCORES = 8
BPC = B // NCORES          # batches per core
ST = T // 128              # 8 key tiles of 128
TB = T // 512              # 2 moving-dim blocks of 512

DT_MM_NAME = os.environ.get("BASS_KERNEL_DT", "bf16")
_DT_MAP = {"bf16": mybir.dt.bfloat16, "f32r": mybir.dt.float32r, "f32": F32}

_CACHE = {}


def _build_nc(dt_mm):
    nc = bacc.Bacc("TRN2", target_bir_lowering=False, debug=False)

    xT = nc.declare_dram_parameter("xT", [BPC, EMB, T], dt_mm, isOutput=False)
    wkn = nc.declare_dram_parameter("wkN", [KH, EMB], dt_mm, isOutput=False)
    wqn = nc.declare_dram_parameter("wqN", [KH, EMB], dt_mm, isOutput=False)
    wv = nc.declare_dram_parameter("wvT", [EMB, KH], dt_mm, isOutput=False)
    pkw = nc.declare_dram_parameter("pkwT", [BPC, KH, KH], dt_mm, isOutput=False)
    pqw = nc.declare_dram_parameter("pqwT", [BPC, KH, KH], dt_mm, isOutput=False)
    pkb = nc.declare_dram_parameter("pkb", [BPC, KH], F32, isOutput=False)
    pqb = nc.declare_dram_parameter("pqb", [BPC, KH], F32, isOutput=False)
    mb = nc.declare_dram_parameter("mb", [BPC, T], F32, isOutput=False)
    out = nc.declare_dram_parameter("out", [BPC, T, KH], F32, isOutput=True)
    dscratch = nc.dram_tensor("dscratch", [BPC, T], F32)

    with TileContext(nc) as tc:
        with tc.tile_pool(name="const", bufs=1) as cpool, \
             tc.tile_pool(name="work", bufs=2) as wpool, \
             tc.tile_pool(name="big", bufs=1) as bpool, \
             tc.tile_pool(name="psa", bufs=2, space="PSUM") as psa, \
             tc.tile_pool(name="psd", bufs=2, space="PSUM") as psdp, \
             tc.tile_pool(name="psx", bufs=2, space="PSUM") as psxp:

            # Wk/Wq natural [h, e] as 2 h-tiles side by side; WvT [e, d]
            wknt = cpool.tile([128, 2 * EMB], dt_mm, name="wknt")
            wqnt = cpool.tile([128, 2 * EMB], dt_mm, name="wqnt")
            wvt = cpool.tile([128, KH], dt_mm, name="wvt")
            ones = cpool.tile([128, 1], dt_mm, name="ones")
            nc.vector.memset(ones, 1.0)
            for hh in range(2):
                nc.sync.dma_start(out=wknt[:, hh * EMB:(hh + 1) * EMB],
                                  in_=wkn[hh * 128:(hh + 1) * 128, :])
                nc.sync.dma_start(out=wqnt[:, hh * EMB:(hh + 1) * EMB],
                                  in_=wqn[hh * 128:(hh + 1) * 128, :])
            nc.sync.dma_start(out=wvt, in_=wv[:, :])

            for b in range(BPC):
                # ---- load per-batch operands -------------------------------
                xt = wpool.tile([128, T], dt_mm, name="xt")
                nc.sync.dma_start(out=xt, in_=xT[b])

                # PK_W[p].T is [h, o]; rows h0/h1 side by side on free axis
                pkwt = wpool.tile([128, 2 * KH], dt_mm, name="pkwt")
                pqwt = wpool.tile([128, 2 * KH], dt_mm, name="pqwt")
                for hh in range(2):
                    nc.sync.dma_start(out=pkwt[:, hh * KH:(hh + 1) * KH],
                                      in_=pkw[b, hh * 128:(hh + 1) * 128, :])
                    nc.sync.dma_start(out=pqwt[:, hh * KH:(hh + 1) * KH],
                                      in_=pqw[b, hh * 128:(hh + 1) * 128, :])
                pkbt = wpool.tile([128, 2], F32, name="pkbt")
                pqbt = wpool.tile([128, 2], F32, name="pqbt")
                mbt = wpool.tile([128, ST], F32, name="mbt")
                nc.sync.dma_start(out=pkbt, in_=pkb[b].rearrange("(a p) -> p a", p=128))
                nc.sync.dma_start(out=pqbt, in_=pqb[b].rearrange("(a p) -> p a", p=128))
                nc.sync.dma_start(out=mbt, in_=mb[b].rearrange("(a p) -> p a", p=128))

                # ---- fold person matrices into projection weights ---------
                # weffT[e, o] = sum_h W[h, e] * PW_T[h, o]
                wkeff = wpool.tile([128, KH], dt_mm, name="wkeff")
                wqeff = wpool.tile([128, KH], dt_mm, name="wqeff")
                for (wn, pw, weff) in ((wknt, pkwt, wkeff), (wqnt, pqwt, wqeff)):
                    pse = psa.tile([128, KH], F32, name="pse", tag="a")
                    for hh in range(2):
                        nc.tensor.matmul(pse, wn[:, hh * EMB:(hh + 1) * EMB],
                                         pw[:, hh * KH:(hh + 1) * KH],
                                         start=(hh == 0), stop=(hh == 1))
                    nc.scalar.copy(weff, pse)

                # ---- projections ------------------------------------------
                # keysT/queriesT: [o, s] as [128, oh*T + s];  v: [s, d]
                kt = wpool.tile([128, 2 * T], dt_mm, name="kt")
                qt = wpool.tile([128, 2 * T], dt_mm, name="qt")
                vt = wpool.tile([128, ST * KH], dt_mm, name="vt")
                for (weff, bt_, dst) in ((wkeff, pkbt, kt), (wqeff, pqbt, qt)):
                    for oh in range(2):
                        for sb in range(TB):
                            psp = psa.tile([128, 512], F32, name="psp", tag="a")
                            nc.tensor.matmul(psp, weff[:, oh * 128:(oh + 1) * 128],
                                             xt[:, sb * 512:(sb + 1) * 512])
                            nc.vector.tensor_scalar_add(
                                dst[:, oh * T + sb * 512:oh * T + (sb + 1) * 512],
                                psp, bt_[:, oh:oh + 1])
                for st in range(ST):
                    psv = psa.tile([128, KH], F32, name="psv", tag="a")
                    nc.tensor.matmul(psv, xt[:, st * 128:(st + 1) * 128], wvt)
                    nc.scalar.copy(vt[:, st * KH:(st + 1) * KH], psv)

                # ---- attention scores + exp + early denominator tree ------
                # dotT: [s, t]; E[s, t] = exp(dot + maskbias[s])
                et = bpool.tile([128, ST * T], dt_mm, name="et", bufs=1)
                lvl1 = [bpool.tile([128, T], F32, name=f"lvl1_{i}", bufs=1)
                        for i in range(4)]
                lvl2 = [bpool.tile([128, T], F32, name=f"lvl2_{i}", bufs=1)
                        for i in range(2)]
                etot = bpool.tile([128, T], dt_mm, name="etot", bufs=1)
                for st in range(ST):
                    psd = psdp.tile([128, T], F32, name="psd", tag="d")
                    for dh in range(2):
                        lhs = kt[:, dh * T + st * 128:dh * T + (st + 1) * 128]
                        for tb in range(TB):
                            nc.tensor.matmul(
                                psd[:, tb * 512:(tb + 1) * 512], lhs,
                                qt[:, dh * T + tb * 512:dh * T + (tb + 1) * 512],
                                start=(dh == 0), stop=(dh == 1))
                    nc.scalar.activation(et[:, st * T:(st + 1) * T], psd,
                                         AF.Exp, bias=mbt[:, st:st + 1])
                    # pairwise denominator tree, overlapped with later tiles
                    if st % 2 == 1:
                        i = st // 2
                        eng = nc.gpsimd if i % 2 == 0 else nc.vector
                        eng.tensor_add(lvl1[i], et[:, (st - 1) * T:st * T],
                                       et[:, st * T:(st + 1) * T])
                    if st == 3:
                        nc.vector.tensor_add(lvl2[0], lvl1[0], lvl1[1])
                    if st == ST - 1:
                        nc.vector.tensor_add(lvl2[1], lvl1[2], lvl1[3])
                        nc.vector.tensor_add(etot, lvl2[0], lvl2[1])

                # denominator: ones-matmul partition-sum -> [1, T] in PSUM,
                # bounce through DRAM to relayout as [128, ST] (t = st*128+p),
                # then a tiny reciprocal.
                sden = wpool.tile([1, T], F32, name="sden")
                for tb in range(TB):
                    psden = psa.tile([1, 512], F32, name="psden", tag="a")
                    nc.tensor.matmul(psden, ones,
                                     etot[:, tb * 512:(tb + 1) * 512])
                    nc.scalar.copy(sden[:, tb * 512:(tb + 1) * 512], psden)
                nc.scalar.dma_start(out=dscratch[b], in_=sden)
                dsmall = wpool.tile([128, ST], F32, name="dsmall")
                nc.scalar.dma_start(out=dsmall,
                                    in_=dscratch[b].rearrange("(a p) -> p a", p=128))
                rcp8 = wpool.tile([128, ST], F32, name="rcp8")
                nc.vector.reciprocal(rcp8, dsmall)

                # ---- context: ctx[t, d] = sum_s E[s, t] * v[s, d] ---------
                # lhsT = E tile slice (stationary), rhs = v tile; output is in
                # natural [t, d] layout so the softmax normalize is a
                # per-partition tensor_scalar and the DMA out needs no
                # transpose anywhere.
                for tt in range(ST):
                    psx = psxp.tile([128, KH], F32, name="psx", tag="x")
                    for st in range(ST):
                        nc.tensor.matmul(
                            psx, et[:, st * T + tt * 128:st * T + (tt + 1) * 128],
                            vt[:, st * KH:(st + 1) * KH],
                            start=(st == 0), stop=(st == ST - 1))
                    ctxn = wpool.tile([128, KH], F32, name="ctxn")
                    nc.vector.tensor_scalar_mul(ctxn, psx, rcp8[:, tt:tt + 1])
                    nc.scalar.dma_start(out=out[b, tt * 128:(tt + 1) * 128, :],
                                        in_=ctxn)

    nc.compile()
    return nc


def _get_nc():
    if "nc" not in _CACHE:
        _CACHE["nc"] = _build_nc(_DT_MAP[DT_MM_NAME])
    return _CACHE["nc"]


def _np_mm_dtype():
    if DT_MM_NAME == "bf16":
        import ml_dtypes
        return np.dtype(ml_dtypes.bfloat16)
    return np.float32


def build_in_maps(x, mask, person_idxs, Wk, Wq, Wv, PK_W, PK_b, PQ_W, PQ_b):
    x = np.asarray(x, dtype=np.float32)
    mask = np.asarray(mask)
    idx = np.asarray(person_idxs).astype(np.int64)
    sk = 1.0 / math.sqrt(KH)
    mdt = _np_mm_dtype()

    wkN = np.ascontiguousarray(np.asarray(Wk, np.float32)).astype(mdt)   # [KH, EMB]
    wqN = np.ascontiguousarray(np.asarray(Wq, np.float32)).astype(mdt)
    wvT = np.ascontiguousarray(np.asarray(Wv, np.float32).T).astype(mdt)
    mbias = np.where(mask[:, 0, :], 0.0, -30.0).astype(np.float32)  # [B, T]

    in_maps = []
    for c in range(NCORES):
        bs = slice(c * BPC, (c + 1) * BPC)
        ci = idx[bs]
        in_maps.append({
            "xT": np.ascontiguousarray(x[bs].transpose(0, 2, 1)).astype(mdt),
            "wkN": wkN, "wqN": wqN, "wvT": wvT,
            "pkwT": np.ascontiguousarray(
                np.asarray(PK_W, np.float32)[ci].transpose(0, 2, 1)).astype(mdt),
            "pqwT": np.ascontiguousarray(
                (np.asarray(PQ_W, np.float32)[ci] * sk).transpose(0, 2, 1)).astype(mdt),
            "pkb": np.ascontiguousarray(np.asarray(PK_b, np.float32)[ci]),
            "pqb": np.ascontiguousarray(np.asarray(PQ_b, np.float32)[ci] * sk),
            "mb": np.ascontiguousarray(mbias[bs]),
        })
    return in_maps


def kernel(x, mask, person_idxs, Wk, Wq, Wv, PK_W, PK_b, PQ_W, PQ_b):
    in_maps = build_in_maps(x, mask, person_idxs, Wk, Wq, Wv, PK_W, PK_b, PQ_W, PQ_b)
    nc = _get_nc()
    res = run_bass_kernel_spmd(nc, in_maps, list(range(NCORES)))
    return np.concatenate([res.results[c]["out"] for c in range(NCORES)], axis=0)


# revision 12
# speedup vs baseline: 1.0378x; 1.0378x over previous
"""Trainium2 Bass kernel for PersonalizedSimpleAttention.

Computation (per batch b, person p = person_idxs[b]):
    keys    = x @ (PK_W[p] @ Wk).T + PK_b[p]               # folded projection
    queries = x @ (PQ_W[p] @ Wq / sqrt(KH)).T + PQ_b[p] / sqrt(KH)
    v       = x @ Wv.T
    attn    = softmax(queries @ keys.T + maskbias, axis=-1)
    out     = attn @ v                                     # [T, VH]

The personalized [KH,KH] @ [KH,EMB] fold happens on-device (4 small matmuls
per batch) and removes the k0/q0 intermediates of the naive two-stage form.

Sharding: data-parallel over batch across 8 cores (8 batches each); the
per-person weight stacks are gathered on the host (pure indexing) so each
core receives exactly its 8 weight matrices.  All on-device layouts are
transposed ([feature, token]) so every matmul contracts over the partition
dim with no on-device transposes; softmax runs over the partition (key) dim
via a pairwise DVE/GPSIMD adder tree + gpsimd partition_all_reduce, and the
normalization is folded in after the attn@v matmul (divide by denom once on
[KH, T] instead of on [T, T]).

Matmul operand dtype is selectable (bf16 default; f32r = TF32-like; f32)
with fp32 PSUM accumulation throughout.
"""
import math
import os

import numpy as np

import concourse.bass as bass  # noqa: F401  (registers engines)
import concourse.mybir as mybir
from concourse import bacc
from concourse.bass_utils import run_bass_kernel_spmd
from concourse.tile import TileContext

F32 = mybir.dt.float32
AF = mybir.ActivationFunctionType

B, T, EMB, KH = 64, 1024, 128, 256
NCORES = 8
BPC = B // NCORES          # batches per core
ST = T // 128              # 8 key tiles of 128
TB = T // 512              # 2 moving-dim blocks of 512

DT_MM_NAME = os.environ.get("BASS_KERNEL_DT", "bf16")
_DT_MAP = {"bf16": mybir.dt.bfloat16, "f32r": mybir.dt.float32r, "f32": F32}

_CACHE = {}


def _build_nc(dt_mm):
    nc = bacc.Bacc("TRN2", target_bir_lowering=False, debug=False)

    xT = nc.declare_dram_parameter("xT", [BPC, EMB, T], dt_mm, isOutput=False)
    wkn = nc.declare_dram_parameter("wkN", [KH, EMB], dt_mm, isOutput=False)
    wqn = nc.declare_dram_parameter("wqN", [KH, EMB], dt_mm, isOutput=False)
    wv = nc.declare_dram_parameter("wvT", [EMB, KH], dt_mm, isOutput=False)
    pkw = nc.declare_dram_parameter("pkwT", [BPC, KH, KH], dt_mm, isOutput=False)
    pqw = nc.declare_dram_parameter("pqwT", [BPC, KH, KH], dt_mm, isOutput=False)
    pkb = nc.declare_dram_parameter("pkb", [BPC, KH], F32, isOutput=False)
    pqb = nc.declare_dram_parameter("pqb", [BPC, KH], F32, isOutput=False)
    mb = nc.declare_dram_parameter("mb", [BPC, T], F32, isOutput=False)
    out = nc.declare_dram_parameter("out", [BPC, T, KH], F32, isOutput=True)
    dscratch = nc.dram_tensor("dscratch", [BPC, T], F32)

    with TileContext(nc) as tc:
        with tc.tile_pool(name="const", bufs=1) as cpool, \
             tc.tile_pool(name="work", bufs=3) as wpool, \
             tc.tile_pool(name="big", bufs=1) as bpool, \
             tc.tile_pool(name="psa", bufs=2, space="PSUM") as psa, \
             tc.tile_pool(name="psd", bufs=2, space="PSUM") as psdp, \
             tc.tile_pool(name="psx", bufs=2, space="PSUM") as psxp:

            # Wk/Wq natural [h, e] as 2 h-tiles side by side; WvT [e, d]
            wknt = cpool.tile([128, 2 * EMB], dt_mm, name="wknt")
            wqnt = cpool.tile([128, 2 * EMB], dt_mm, name="wqnt")
            wvt = cpool.tile([128, KH], dt_mm, name="wvt")
            ones = cpool.tile([128, 1], dt_mm, name="ones")
            nc.vector.memset(ones, 1.0)
            for hh in range(2):
                nc.sync.dma_start(out=wknt[:, hh * EMB:(hh + 1) * EMB],
                                  in_=wkn[hh * 128:(hh + 1) * 128, :])
                nc.sync.dma_start(out=wqnt[:, hh * EMB:(hh + 1) * EMB],
                                  in_=wqn[hh * 128:(hh + 1) * 128, :])
            nc.sync.dma_start(out=wvt, in_=wv[:, :])

            for b in range(BPC):
                # ---- load per-batch operands -------------------------------
                xt = wpool.tile([128, T], dt_mm, name="xt")
                nc.sync.dma_start(out=xt, in_=xT[b])

                # PK_W[p].T is [h, o]; rows h0/h1 side by side on free axis
                pkwt = wpool.tile([128, 2 * KH], dt_mm, name="pkwt")
                pqwt = wpool.tile([128, 2 * KH], dt_mm, name="pqwt")
                for hh in range(2):
                    nc.sync.dma_start(out=pkwt[:, hh * KH:(hh + 1) * KH],
                                      in_=pkw[b, hh * 128:(hh + 1) * 128, :])
                    nc.sync.dma_start(out=pqwt[:, hh * KH:(hh + 1) * KH],
                                      in_=pqw[b, hh * 128:(hh + 1) * 128, :])
                pkbt = wpool.tile([128, 2], F32, name="pkbt")
                pqbt = wpool.tile([128, 2], F32, name="pqbt")
                mbt = wpool.tile([128, ST], F32, name="mbt")
                nc.sync.dma_start(out=pkbt, in_=pkb[b].rearrange("(a p) -> p a", p=128))
                nc.sync.dma_start(out=pqbt, in_=pqb[b].rearrange("(a p) -> p a", p=128))
                nc.sync.dma_start(out=mbt, in_=mb[b].rearrange("(a p) -> p a", p=128))

                # ---- fold person matrices into projection weights ---------
                # weffT[e, o] = sum_h W[h, e] * PW_T[h, o]
                wkeff = wpool.tile([128, KH], dt_mm, name="wkeff")
                wqeff = wpool.tile([128, KH], dt_mm, name="wqeff")
                for (wn, pw, weff) in ((wknt, pkwt, wkeff), (wqnt, pqwt, wqeff)):
                    pse = psa.tile([128, KH], F32, name="pse", tag="a")
                    for hh in range(2):
                        nc.tensor.matmul(pse, wn[:, hh * EMB:(hh + 1) * EMB],
                                         pw[:, hh * KH:(hh + 1) * KH],
                                         start=(hh == 0), stop=(hh == 1))
                    nc.scalar.copy(weff, pse)

                # ---- projections ------------------------------------------
                # keysT/queriesT: [o, s] as [128, oh*T + s];  v: [s, d]
                kt = wpool.tile([128, 2 * T], dt_mm, name="kt")
                qt = wpool.tile([128, 2 * T], dt_mm, name="qt")
                vt = wpool.tile([128, ST * KH], dt_mm, name="vt")
                for (weff, bt_, dst) in ((wkeff, pkbt, kt), (wqeff, pqbt, qt)):
                    for oh in range(2):
                        for sb in range(TB):
                            psp = psa.tile([128, 512], F32, name="psp", tag="a")
                            nc.tensor.matmul(psp, weff[:, oh * 128:(oh + 1) * 128],
                                             xt[:, sb * 512:(sb + 1) * 512])
                            nc.vector.tensor_scalar_add(
                                dst[:, oh * T + sb * 512:oh * T + (sb + 1) * 512],
                                psp, bt_[:, oh:oh + 1])
                for st in range(ST):
                    psv = psa.tile([128, KH], F32, name="psv", tag="a")
                    nc.tensor.matmul(psv, xt[:, st * 128:(st + 1) * 128], wvt)
                    nc.scalar.copy(vt[:, st * KH:(st + 1) * KH], psv)

                # ---- attention scores + exp + early denominator tree ------
                # dotT: [s, t]; E[s, t] = exp(dot + maskbias[s])
                et = bpool.tile([128, ST * T], dt_mm, name="et", bufs=1)
                lvl1 = [bpool.tile([128, T], F32, name=f"lvl1_{i}", bufs=1)
                        for i in range(4)]
                lvl2 = [bpool.tile([128, T], F32, name=f"lvl2_{i}", bufs=1)
                        for i in range(2)]
                etot = bpool.tile([128, T], dt_mm, name="etot", bufs=1)
                for st in range(ST):
                    psd = psdp.tile([128, T], F32, name="psd", tag="d")
                    for dh in range(2):
                        lhs = kt[:, dh * T + st * 128:dh * T + (st + 1) * 128]
                        for tb in range(TB):
                            nc.tensor.matmul(
                                psd[:, tb * 512:(tb + 1) * 512], lhs,
                                qt[:, dh * T + tb * 512:dh * T + (tb + 1) * 512],
                                start=(dh == 0), stop=(dh == 1))
                    nc.scalar.activation(et[:, st * T:(st + 1) * T], psd,
                                         AF.Exp, bias=mbt[:, st:st + 1])
                    # pairwise denominator tree, overlapped with later tiles
                    if st % 2 == 1:
                        i = st // 2
                        eng = nc.gpsimd if i % 2 == 0 else nc.vector
                        eng.tensor_add(lvl1[i], et[:, (st - 1) * T:st * T],
                                       et[:, st * T:(st + 1) * T])
                    if st == 3:
                        nc.vector.tensor_add(lvl2[0], lvl1[0], lvl1[1])
                    if st == ST - 1:
                        nc.vector.tensor_add(lvl2[1], lvl1[2], lvl1[3])
                        nc.vector.tensor_add(etot, lvl2[0], lvl2[1])

                # denominator: ones-matmul partition-sum -> [1, T] in PSUM,
                # bounce through DRAM to relayout as [128, ST] (t = st*128+p),
                # then a tiny reciprocal.
                sden = wpool.tile([1, T], F32, name="sden")
                for tb in range(TB):
                    psden = psa.tile([1, 512], F32, name="psden", tag="a")
                    nc.tensor.matmul(psden, ones,
                                     etot[:, tb * 512:(tb + 1) * 512])
                    nc.scalar.copy(sden[:, tb * 512:(tb + 1) * 512], psden)
                nc.sync.dma_start(out=dscratch[b], in_=sden)
                dsmall = wpool.tile([128, ST], F32, name="dsmall")
                nc.sync.dma_start(out=dsmall,
                                  in_=dscratch[b].rearrange("(a p) -> p a", p=128))
                rcp8 = wpool.tile([128, ST], F32, name="rcp8")
                nc.vector.reciprocal(rcp8, dsmall)

                # ---- context: ctx[t, d] = sum_s E[s, t] * v[s, d] ---------
                # lhsT = E tile slice (stationary), rhs = v tile; output is in
                # natural [t, d] layout so the softmax normalize is a
                # per-partition tensor_scalar and the DMA out needs no
                # transpose anywhere.
                for tt in range(ST):
                    psx = psxp.tile([128, KH], F32, name="psx", tag="x")
                    for st in range(ST):
                        nc.tensor.matmul(
                            psx, et[:, st * T + tt * 128:st * T + (tt + 1) * 128],
                            vt[:, st * KH:(st + 1) * KH],
                            start=(st == 0), stop=(st == ST - 1))
                    ctxn = wpool.tile([128, KH], F32, name="ctxn")
                    nc.vector.tensor_scalar_mul(ctxn, psx, rcp8[:, tt:tt + 1])
                    nc.sync.dma_start(out=out[b, tt * 128:(tt + 1) * 128, :],
                                      in_=ctxn)

    nc.compile()
    return nc


def _get_nc():
    if "nc" not in _CACHE:
        _CACHE["nc"] = _build_nc(_DT_MAP[DT_MM_NAME])
    return _CACHE["nc"]


def _np_mm_dtype():
    if DT_MM_NAME == "bf16":
        import ml_dtypes
        return np.dtype(ml_dtypes.bfloat16)
    return np.float32


def build_in_maps(x, mask, person_idxs, Wk, Wq, Wv, PK_W, PK_b, PQ_W, PQ_b):
    x = np.asarray(x, dtype=np.float32)
    mask = np.asarray(mask)
    idx = np.asarray(person_idxs).astype(np.int64)
    sk = 1.0 / math.sqrt(KH)
    mdt = _np_mm_dtype()

    wkN = np.ascontiguousarray(np.asarray(Wk, np.float32)).astype(mdt)   # [KH, EMB]
    wqN = np.ascontiguousarray(np.asarray(Wq, np.float32)).astype(mdt)
    wvT = np.ascontiguousarray(np.asarray(Wv, np.float32).T).astype(mdt)
    mbias = np.where(mask[:, 0, :], 0.0, -30.0).astype(np.float32)  # [B, T]

    in_maps = []
    for c in range(NCORES):
        bs = slice(c * BPC, (c + 1) * BPC)
        ci = idx[bs]
        in_maps.append({
            "xT": np.ascontiguousarray(x[bs].transpose(0, 2, 1)).astype(mdt),
            "wkN": wkN, "wqN": wqN, "wvT": wvT,
            "pkwT": np.ascontiguousarray(
                np.asarray(PK_W, np.float32)[ci].transpose(0, 2, 1)).astype(mdt),
            "pqwT": np.ascontiguousarray(
                (np.asarray(PQ_W, np.float32)[ci] * sk).transpose(0, 2, 1)).astype(mdt),
            "pkb": np.ascontiguousarray(np.asarray(PK_b, np.float32)[ci]),
            "pqb": np.ascontiguousarray(np.asarray(PQ_b, np.float32)[ci] * sk),
            "mb": np.ascontiguousarray(mbias[bs]),
        })
    return in_maps


def kernel(x, mask, person_idxs, Wk, Wq, Wv, PK_W, PK_b, PQ_W, PQ_b):
    in_maps = build_in_maps(x, mask, person_idxs, Wk, Wq, Wv, PK_W, PK_b, PQ_W, PQ_b)
    nc = _get_nc()
    res = run_bass_kernel_spmd(nc, in_maps, list(range(NCORES)))
    return np.concatenate([res.results[c]["out"] for c in range(NCORES)], axis=0)


# revision 14
# speedup vs baseline: 1.4118x; 1.3604x over previous
"""Trainium2 Bass kernel for PersonalizedSimpleAttention.

Computation (per batch b, person p = person_idxs[b]):
    keys    = x @ (PK_W[p] @ Wk).T + PK_b[p]               # folded projection
    queries = x @ (PQ_W[p] @ Wq / sqrt(KH)).T + PQ_b[p] / sqrt(KH)
    v       = x @ Wv.T
    attn    = softmax(queries @ keys.T + maskbias, axis=-1)
    out     = attn @ v                                     # [T, VH]

The personalized [KH,KH] @ [KH,EMB] fold happens on-device (4 small matmuls
per batch) and removes the k0/q0 intermediates of the naive two-stage form.

Sharding: data-parallel over batch across 8 cores (8 batches each); the
per-person weight stacks are gathered on the host (pure indexing) so each
core receives exactly its 8 weight matrices.  All on-device layouts are
transposed ([feature, token]) so every matmul contracts over the partition
dim with no on-device transposes; softmax runs over the partition (key) dim
via a pairwise DVE/GPSIMD adder tree + gpsimd partition_all_reduce, and the
normalization is folded in after the attn@v matmul (divide by denom once on
[KH, T] instead of on [T, T]).

Matmul operand dtype is selectable (bf16 default; f32r = TF32-like; f32)
with fp32 PSUM accumulation throughout.
"""
import math
import os

import numpy as np

import concourse.bass as bass  # noqa: F401  (registers engines)
import concourse.mybir as mybir
from concourse import bacc
from concourse.bass_utils import run_bass_kernel_spmd
from concourse.tile import TileContext

F32 = mybir.dt.float32
AF = mybir.ActivationFunctionType

B, T, EMB, KH = 64, 1024, 128, 256
NCORES = 8
BPC = B // NCORES          # batches per core
ST = T // 128              # 8 key tiles of 128
TB = T // 512              # 2 moving-dim blocks of 512

DT_MM_NAME = os.environ.get("BASS_KERNEL_DT", "bf16")
_DT_MAP = {"bf16": mybir.dt.bfloat16, "f32r": mybir.dt.float32r, "f32": F32}

_CACHE = {}


def _build_nc(dt_mm):
    nc = bacc.Bacc("TRN2", target_bir_lowering=False, debug=False)

    xT = nc.declare_dram_parameter("xT", [BPC, EMB, T], dt_mm, isOutput=False)
    wkn = nc.declare_dram_parameter("wkN", [KH, EMB], dt_mm, isOutput=False)
    wqn = nc.declare_dram_parameter("wqN", [KH, EMB], dt_mm, isOutput=False)
    wv = nc.declare_dram_parameter("wvT", [EMB, KH], dt_mm, isOutput=False)
    pkw = nc.declare_dram_parameter("pkwT", [BPC, KH, KH], dt_mm, isOutput=False)
    pqw = nc.declare_dram_parameter("pqwT", [BPC, KH, KH], dt_mm, isOutput=False)
    pkb = nc.declare_dram_parameter("pkb", [BPC, KH], F32, isOutput=False)
    pqb = nc.declare_dram_parameter("pqb", [BPC, KH], F32, isOutput=False)
    mb = nc.declare_dram_parameter("mb", [BPC, T], F32, isOutput=False)
    out = nc.declare_dram_parameter("out", [BPC, T, KH], F32, isOutput=True)
    KH1 = KH + 1  # v gets a ones column: attn @ [v | 1] yields the softmax denom

    with TileContext(nc) as tc:
        with tc.tile_pool(name="const", bufs=1) as cpool, \
             tc.tile_pool(name="work", bufs=3) as wpool, \
             tc.tile_pool(name="big", bufs=1) as bpool, \
             tc.tile_pool(name="psa", bufs=2, space="PSUM") as psa, \
             tc.tile_pool(name="psd", bufs=2, space="PSUM") as psdp, \
             tc.tile_pool(name="psx", bufs=2, space="PSUM") as psxp:

            # Wk/Wq natural [h, e] as 2 h-tiles side by side; WvT [e, d]
            wknt = cpool.tile([128, 2 * EMB], dt_mm, name="wknt")
            wqnt = cpool.tile([128, 2 * EMB], dt_mm, name="wqnt")
            wvt = cpool.tile([128, KH], dt_mm, name="wvt")
            for hh in range(2):
                nc.sync.dma_start(out=wknt[:, hh * EMB:(hh + 1) * EMB],
                                  in_=wkn[hh * 128:(hh + 1) * 128, :])
                nc.sync.dma_start(out=wqnt[:, hh * EMB:(hh + 1) * EMB],
                                  in_=wqn[hh * 128:(hh + 1) * 128, :])
            nc.sync.dma_start(out=wvt, in_=wv[:, :])

            # ---- fold person matrices into projection weights (all batches
            # upfront, so the steady-state loop never waits on this chain) ---
            # weffT[e, o] = sum_h W[h, e] * PW_T[h, o]
            wkeffs = [cpool.tile([128, KH], dt_mm, name=f"wkeff{b}") for b in range(BPC)]
            wqeffs = [cpool.tile([128, KH], dt_mm, name=f"wqeff{b}") for b in range(BPC)]
            with tc.tile_pool(name="pw", bufs=2) as pwpool:
                for b in range(BPC):
                    for (wn, pw_d, weff) in ((wknt, pkw, wkeffs[b]), (wqnt, pqw, wqeffs[b])):
                        pwt = pwpool.tile([128, 2 * KH], dt_mm, name="pwt")
                        for hh in range(2):
                            nc.sync.dma_start(out=pwt[:, hh * KH:(hh + 1) * KH],
                                              in_=pw_d[b, hh * 128:(hh + 1) * 128, :])
                        pse = psa.tile([128, KH], F32, name="pse", tag="a")
                        for hh in range(2):
                            nc.tensor.matmul(pse, wn[:, hh * EMB:(hh + 1) * EMB],
                                             pwt[:, hh * KH:(hh + 1) * KH],
                                             start=(hh == 0), stop=(hh == 1))
                        nc.scalar.copy(weff, pse)

            for b in range(BPC):
                # ---- load per-batch operands -------------------------------
                xt = wpool.tile([128, T], dt_mm, name="xt")
                nc.sync.dma_start(out=xt, in_=xT[b])
                pkbt = wpool.tile([128, 2], F32, name="pkbt")
                pqbt = wpool.tile([128, 2], F32, name="pqbt")
                mbt = wpool.tile([128, ST], F32, name="mbt")
                nc.sync.dma_start(out=pkbt, in_=pkb[b].rearrange("(a p) -> p a", p=128))
                nc.sync.dma_start(out=pqbt, in_=pqb[b].rearrange("(a p) -> p a", p=128))
                nc.sync.dma_start(out=mbt, in_=mb[b].rearrange("(a p) -> p a", p=128))

                # ---- projections ------------------------------------------
                # keysT/queriesT: [o, s] as [128, oh*T + s]
                # v (with ones column): [s, d] as [128, st*KH1 + d], col KH = 1.0
                kt = wpool.tile([128, 2 * T], dt_mm, name="kt")
                qt = wpool.tile([128, 2 * T], dt_mm, name="qt")
                vt = wpool.tile([128, ST * KH1], dt_mm, name="vt")
                for (weff, bt_, dst) in ((wkeffs[b], pkbt, kt), (wqeffs[b], pqbt, qt)):
                    for oh in range(2):
                        for sb in range(TB):
                            psp = psa.tile([128, 512], F32, name="psp", tag="a")
                            nc.tensor.matmul(psp, weff[:, oh * 128:(oh + 1) * 128],
                                             xt[:, sb * 512:(sb + 1) * 512])
                            nc.vector.tensor_scalar_add(
                                dst[:, oh * T + sb * 512:oh * T + (sb + 1) * 512],
                                psp, bt_[:, oh:oh + 1])
                nc.vector.memset(
                    vt.rearrange("p (st c) -> p st c", c=KH1)[:, :, KH:KH1], 1.0)
                for st in range(ST):
                    psv = psa.tile([128, KH], F32, name="psv", tag="a")
                    nc.tensor.matmul(psv, xt[:, st * 128:(st + 1) * 128], wvt)
                    nc.scalar.copy(vt[:, st * KH1:st * KH1 + KH], psv)

                # ---- attention scores + exp -------------------------------
                # dotT: [s, t]; E[s, t] = exp(dot + maskbias[s])
                et = bpool.tile([128, ST * T], dt_mm, name="et", bufs=1)
                for st in range(ST):
                    psd = psdp.tile([128, T], F32, name="psd", tag="d")
                    for dh in range(2):
                        lhs = kt[:, dh * T + st * 128:dh * T + (st + 1) * 128]
                        for tb in range(TB):
                            nc.tensor.matmul(
                                psd[:, tb * 512:(tb + 1) * 512], lhs,
                                qt[:, dh * T + tb * 512:dh * T + (tb + 1) * 512],
                                start=(dh == 0), stop=(dh == 1))
                    nc.scalar.activation(et[:, st * T:(st + 1) * T], psd,
                                         AF.Exp, bias=mbt[:, st:st + 1])

                # ---- context + softmax denominator in one matmul ----------
                # ctx_aug[t, 0:KH]  = sum_s E[s, t] * v[s, d]
                # ctx_aug[t, KH]    = sum_s E[s, t]          (the ones column)
                # Output lands in natural [t, d] layout; normalization is a
                # per-partition reciprocal+scale, no transpose anywhere.
                for tt in range(ST):
                    psx = psxp.tile([128, KH1], F32, name="psx", tag="x")
                    for st in range(ST):
                        nc.tensor.matmul(
                            psx, et[:, st * T + tt * 128:st * T + (tt + 1) * 128],
                            vt[:, st * KH1:(st + 1) * KH1],
                            start=(st == 0), stop=(st == ST - 1))
                    rcpc = wpool.tile([128, 1], F32, name="rcpc")
                    nc.vector.reciprocal(rcpc, psx[:, KH:KH1])
                    ctxn = wpool.tile([128, KH], F32, name="ctxn")
                    nc.vector.tensor_scalar_mul(ctxn, psx[:, 0:KH], rcpc)
                    nc.sync.dma_start(out=out[b, tt * 128:(tt + 1) * 128, :],
                                      in_=ctxn)

    nc.compile()
    return nc


def _get_nc():
    if "nc" not in _CACHE:
        _CACHE["nc"] = _build_nc(_DT_MAP[DT_MM_NAME])
    return _CACHE["nc"]


def _np_mm_dtype():
    if DT_MM_NAME == "bf16":
        import ml_dtypes
        return np.dtype(ml_dtypes.bfloat16)
    return np.float32


def build_in_maps(x, mask, person_idxs, Wk, Wq, Wv, PK_W, PK_b, PQ_W, PQ_b):
    x = np.asarray(x, dtype=np.float32)
    mask = np.asarray(mask)
    idx = np.asarray(person_idxs).astype(np.int64)
    sk = 1.0 / math.sqrt(KH)
    mdt = _np_mm_dtype()

    wkN = np.ascontiguousarray(np.asarray(Wk, np.float32)).astype(mdt)   # [KH, EMB]
    wqN = np.ascontiguousarray(np.asarray(Wq, np.float32)).astype(mdt)
    wvT = np.ascontiguousarray(np.asarray(Wv, np.float32).T).astype(mdt)
    mbias = np.where(mask[:, 0, :], 0.0, -30.0).astype(np.float32)  # [B, T]

    in_maps = []
    for c in range(NCORES):
        bs = slice(c * BPC, (c + 1) * BPC)
        ci = idx[bs]
        in_maps.append({
            "xT": np.ascontiguousarray(x[bs].transpose(0, 2, 1)).astype(mdt),
            "wkN": wkN, "wqN": wqN, "wvT": wvT,
            "pkwT": np.ascontiguousarray(
                np.asarray(PK_W, np.float32)[ci].transpose(0, 2, 1)).astype(mdt),
            "pqwT": np.ascontiguousarray(
                (np.asarray(PQ_W, np.float32)[ci] * sk).transpose(0, 2, 1)).astype(mdt),
            "pkb": np.ascontiguousarray(np.asarray(PK_b, np.float32)[ci]),
            "pqb": np.ascontiguousarray(np.asarray(PQ_b, np.float32)[ci] * sk),
            "mb": np.ascontiguousarray(mbias[bs]),
        })
    return in_maps


def kernel(x, mask, person_idxs, Wk, Wq, Wv, PK_W, PK_b, PQ_W, PQ_b):
    in_maps = build_in_maps(x, mask, person_idxs, Wk, Wq, Wv, PK_W, PK_b, PQ_W, PQ_b)
    nc = _get_nc()
    res = run_bass_kernel_spmd(nc, in_maps, list(range(NCORES)))
    return np.concatenate([res.results[c]["out"] for c in range(NCORES)], axis=0)


# revision 16
# speedup vs baseline: 1.4668x; 1.0390x over previous
"""Trainium2 Bass kernel for PersonalizedSimpleAttention.

Computation (per batch b, person p = person_idxs[b]):
    keys    = x @ (PK_W[p] @ Wk).T + PK_b[p]               # folded projection
    queries = x @ (PQ_W[p] @ Wq / sqrt(KH)).T + PQ_b[p] / sqrt(KH)
    v       = x @ Wv.T
    attn    = softmax(queries @ keys.T + maskbias, axis=-1)
    out     = attn @ v                                     # [T, VH]

The personalized [KH,KH] @ [KH,EMB] fold happens on-device (4 small matmuls
per batch) and removes the k0/q0 intermediates of the naive two-stage form.

Sharding: data-parallel over batch across 8 cores (8 batches each); the
per-person weight stacks are gathered on the host (pure indexing) so each
core receives exactly its 8 weight matrices.  All on-device layouts are
transposed ([feature, token]) so every matmul contracts over the partition
dim with no on-device transposes; softmax runs over the partition (key) dim
via a pairwise DVE/GPSIMD adder tree + gpsimd partition_all_reduce, and the
normalization is folded in after the attn@v matmul (divide by denom once on
[KH, T] instead of on [T, T]).

Matmul operand dtype is selectable (bf16 default; f32r = TF32-like; f32)
with fp32 PSUM accumulation throughout.
"""
import math
import os

import numpy as np

import concourse.bass as bass  # noqa: F401  (registers engines)
import concourse.mybir as mybir
from concourse import bacc
from concourse.bass_utils import run_bass_kernel_spmd
from concourse.tile import TileContext

F32 = mybir.dt.float32
AF = mybir.ActivationFunctionType

B, T, EMB, KH = 64, 1024, 128, 256
NCORES = 8
BPC = B // NCORES          # batches per core
ST = T // 128              # 8 key tiles of 128
TB = T // 512              # 2 moving-dim blocks of 512

DT_MM_NAME = os.environ.get("BASS_KERNEL_DT", "bf16")
_DT_MAP = {"bf16": mybir.dt.bfloat16, "f32r": mybir.dt.float32r, "f32": F32}

_CACHE = {}


def _build_nc(dt_mm):
    nc = bacc.Bacc("TRN2", target_bir_lowering=False, debug=False)

    xT = nc.declare_dram_parameter("xT", [BPC, EMB, T], dt_mm, isOutput=False)
    wkn = nc.declare_dram_parameter("wkN", [KH, EMB], dt_mm, isOutput=False)
    wqn = nc.declare_dram_parameter("wqN", [KH, EMB], dt_mm, isOutput=False)
    wv = nc.declare_dram_parameter("wvT", [EMB, KH], dt_mm, isOutput=False)
    pkw = nc.declare_dram_parameter("pkwT", [BPC, KH, KH], dt_mm, isOutput=False)
    pqw = nc.declare_dram_parameter("pqwT", [BPC, KH, KH], dt_mm, isOutput=False)
    pkb = nc.declare_dram_parameter("pkb", [BPC, KH], F32, isOutput=False)
    pqb = nc.declare_dram_parameter("pqb", [BPC, KH], F32, isOutput=False)
    mb = nc.declare_dram_parameter("mb", [BPC, T], F32, isOutput=False)
    out = nc.declare_dram_parameter("out", [BPC, T, KH], F32, isOutput=True)
    KH1 = KH + 1  # v gets a ones column: attn @ [v | 1] yields the softmax denom

    with TileContext(nc) as tc:
        with tc.tile_pool(name="const", bufs=1) as cpool, \
             tc.tile_pool(name="work", bufs=3) as wpool, \
             tc.tile_pool(name="big", bufs=1) as bpool, \
             tc.tile_pool(name="psa", bufs=2, space="PSUM") as psa, \
             tc.tile_pool(name="psd", bufs=2, space="PSUM") as psdp, \
             tc.tile_pool(name="psx", bufs=2, space="PSUM") as psxp:

            # Wk/Wq natural [h, e] as 2 h-tiles side by side; WvT [e, d]
            wknt = cpool.tile([128, 2 * EMB], dt_mm, name="wknt")
            wqnt = cpool.tile([128, 2 * EMB], dt_mm, name="wqnt")
            wvt = cpool.tile([128, KH], dt_mm, name="wvt")
            for hh in range(2):
                nc.sync.dma_start(out=wknt[:, hh * EMB:(hh + 1) * EMB],
                                  in_=wkn[hh * 128:(hh + 1) * 128, :])
                nc.sync.dma_start(out=wqnt[:, hh * EMB:(hh + 1) * EMB],
                                  in_=wqn[hh * 128:(hh + 1) * 128, :])
            nc.sync.dma_start(out=wvt, in_=wv[:, :])

            # ---- fold person matrices into projection weights (all batches
            # upfront, so the steady-state loop never waits on this chain) ---
            # weffT[e, o] = sum_h W[h, e] * PW_T[h, o]
            wkeffs = [cpool.tile([128, KH], dt_mm, name=f"wkeff{b}") for b in range(BPC)]
            wqeffs = [cpool.tile([128, KH], dt_mm, name=f"wqeff{b}") for b in range(BPC)]
            with tc.tile_pool(name="pw", bufs=16) as pwpool:
                pwts = []
                for b in range(BPC):
                    for (j, pw_d) in ((0, pkw), (1, pqw)):
                        pwt = pwpool.tile([128, 2 * KH], dt_mm, name=f"pwt{b}_{j}", tag="pwt")
                        eng = nc.sync if (2 * b + j) % 2 == 0 else nc.scalar
                        for hh in range(2):
                            eng.dma_start(out=pwt[:, hh * KH:(hh + 1) * KH],
                                          in_=pw_d[b, hh * 128:(hh + 1) * 128, :])
                        pwts.append(pwt)
                for b in range(BPC):
                    for (j, wn, weff) in ((0, wknt, wkeffs[b]), (1, wqnt, wqeffs[b])):
                        pwt = pwts[2 * b + j]
                        pse = psa.tile([128, KH], F32, name="pse", tag="a")
                        for hh in range(2):
                            nc.tensor.matmul(pse, wn[:, hh * EMB:(hh + 1) * EMB],
                                             pwt[:, hh * KH:(hh + 1) * KH],
                                             start=(hh == 0), stop=(hh == 1))
                        nc.scalar.copy(weff, pse)

            for b in range(BPC):
                # ---- load per-batch operands -------------------------------
                xt = wpool.tile([128, T], dt_mm, name="xt")
                nc.sync.dma_start(out=xt, in_=xT[b])
                pkbt = wpool.tile([128, 2], F32, name="pkbt")
                pqbt = wpool.tile([128, 2], F32, name="pqbt")
                mbt = wpool.tile([128, ST], F32, name="mbt")
                nc.sync.dma_start(out=pkbt, in_=pkb[b].rearrange("(a p) -> p a", p=128))
                nc.sync.dma_start(out=pqbt, in_=pqb[b].rearrange("(a p) -> p a", p=128))
                nc.sync.dma_start(out=mbt, in_=mb[b].rearrange("(a p) -> p a", p=128))

                # ---- projections ------------------------------------------
                # keysT/queriesT: [o, s] as [128, oh*T + s]
                # v (with ones column): [s, d] as [128, st*KH1 + d], col KH = 1.0
                kt = wpool.tile([128, 2 * T], dt_mm, name="kt")
                qt = wpool.tile([128, 2 * T], dt_mm, name="qt")
                vt = wpool.tile([128, ST * KH1], dt_mm, name="vt")
                for (weff, bt_, dst) in ((wkeffs[b], pkbt, kt), (wqeffs[b], pqbt, qt)):
                    for oh in range(2):
                        for sb in range(TB):
                            psp = psa.tile([128, 512], F32, name="psp", tag="a")
                            nc.tensor.matmul(psp, weff[:, oh * 128:(oh + 1) * 128],
                                             xt[:, sb * 512:(sb + 1) * 512])
                            nc.vector.tensor_scalar_add(
                                dst[:, oh * T + sb * 512:oh * T + (sb + 1) * 512],
                                psp, bt_[:, oh:oh + 1])
                nc.vector.memset(
                    vt.rearrange("p (st c) -> p st c", c=KH1)[:, :, KH:KH1], 1.0)
                for st in range(ST):
                    psv = psa.tile([128, KH], F32, name="psv", tag="a")
                    nc.tensor.matmul(psv, xt[:, st * 128:(st + 1) * 128], wvt)
                    nc.scalar.copy(vt[:, st * KH1:st * KH1 + KH], psv)

                # ---- attention scores + exp -------------------------------
                # dotT: [s, t]; E[s, t] = exp(dot + maskbias[s])
                et = bpool.tile([128, ST * T], dt_mm, name="et", bufs=1)
                for st in range(ST):
                    psd = psdp.tile([128, T], F32, name="psd", tag="d")
                    for dh in range(2):
                        lhs = kt[:, dh * T + st * 128:dh * T + (st + 1) * 128]
                        for tb in range(TB):
                            nc.tensor.matmul(
                                psd[:, tb * 512:(tb + 1) * 512], lhs,
                                qt[:, dh * T + tb * 512:dh * T + (tb + 1) * 512],
                                start=(dh == 0), stop=(dh == 1))
                    nc.scalar.activation(et[:, st * T:(st + 1) * T], psd,
                                         AF.Exp, bias=mbt[:, st:st + 1])

                # ---- context + softmax denominator in one matmul ----------
                # ctx_aug[t, 0:KH]  = sum_s E[s, t] * v[s, d]
                # ctx_aug[t, KH]    = sum_s E[s, t]          (the ones column)
                # Output lands in natural [t, d] layout; normalization is a
                # per-partition reciprocal+scale, no transpose anywhere.
                for tt in range(ST):
                    psx = psxp.tile([128, KH1], F32, name="psx", tag="x")
                    for st in range(ST):
                        nc.tensor.matmul(
                            psx, et[:, st * T + tt * 128:st * T + (tt + 1) * 128],
                            vt[:, st * KH1:(st + 1) * KH1],
                            start=(st == 0), stop=(st == ST - 1))
                    rcpc = wpool.tile([128, 1], F32, name="rcpc")
                    nc.vector.reciprocal(rcpc, psx[:, KH:KH1])
                    ctxn = wpool.tile([128, KH], F32, name="ctxn")
                    nc.vector.tensor_scalar_mul(ctxn, psx[:, 0:KH], rcpc)
                    nc.sync.dma_start(out=out[b, tt * 128:(tt + 1) * 128, :],
                                      in_=ctxn)

    nc.compile()
    return nc


def _get_nc():
    if "nc" not in _CACHE:
        _CACHE["nc"] = _build_nc(_DT_MAP[DT_MM_NAME])
    return _CACHE["nc"]


def _np_mm_dtype():
    if DT_MM_NAME == "bf16":
        import ml_dtypes
        return np.dtype(ml_dtypes.bfloat16)
    return np.float32


def build_in_maps(x, mask, person_idxs, Wk, Wq, Wv, PK_W, PK_b, PQ_W, PQ_b):
    x = np.asarray(x, dtype=np.float32)
    mask = np.asarray(mask)
    idx = np.asarray(person_idxs).astype(np.int64)
    sk = 1.0 / math.sqrt(KH)
    mdt = _np_mm_dtype()

    wkN = np.ascontiguousarray(np.asarray(Wk, np.float32)).astype(mdt)   # [KH, EMB]
    wqN = np.ascontiguousarray(np.asarray(Wq, np.float32)).astype(mdt)
    wvT = np.ascontiguousarray(np.asarray(Wv, np.float32).T).astype(mdt)
    mbias = np.where(mask[:, 0, :], 0.0, -30.0).astype(np.float32)  # [B, T]

    in_maps = []
    for c in range(NCORES):
        bs = slice(c * BPC, (c + 1) * BPC)
        ci = idx[bs]
        in_maps.append({
            "xT": np.ascontiguousarray(x[bs].transpose(0, 2, 1)).astype(mdt),
            "wkN": wkN, "wqN": wqN, "wvT": wvT,
            "pkwT": np.ascontiguousarray(
                np.asarray(PK_W, np.float32)[ci].transpose(0, 2, 1)).astype(mdt),
            "pqwT": np.ascontiguousarray(
                (np.asarray(PQ_W, np.float32)[ci] * sk).transpose(0, 2, 1)).astype(mdt),
            "pkb": np.ascontiguousarray(np.asarray(PK_b, np.float32)[ci]),
            "pqb": np.ascontiguousarray(np.asarray(PQ_b, np.float32)[ci] * sk),
            "mb": np.ascontiguousarray(mbias[bs]),
        })
    return in_maps


def kernel(x, mask, person_idxs, Wk, Wq, Wv, PK_W, PK_b, PQ_W, PQ_b):
    in_maps = build_in_maps(x, mask, person_idxs, Wk, Wq, Wv, PK_W, PK_b, PQ_W, PQ_b)
    nc = _get_nc()
    res = run_bass_kernel_spmd(nc, in_maps, list(range(NCORES)))
    return np.concatenate([res.results[c]["out"] for c in range(NCORES)], axis=0)


# revision 21
# speedup vs baseline: 1.5648x; 1.0668x over previous
"""Trainium2 Bass kernel for PersonalizedSimpleAttention.

Computation (per batch b, person p = person_idxs[b]):
    keys    = x @ (PK_W[p] @ Wk).T + PK_b[p]               # folded projection
    queries = x @ (PQ_W[p] @ Wq / sqrt(KH)).T + PQ_b[p] / sqrt(KH)
    v       = x @ Wv.T
    attn    = softmax(queries @ keys.T + maskbias, axis=-1)
    out     = attn @ v                                     # [T, VH]

The personalized [KH,KH] @ [KH,EMB] fold happens on-device (4 small matmuls
per batch) and removes the k0/q0 intermediates of the naive two-stage form.

Sharding: data-parallel over batch across 8 cores (8 batches each); the
per-person weight stacks are gathered on the host (pure indexing) so each
core receives exactly its 8 weight matrices.  All on-device layouts are
transposed ([feature, token]) so every matmul contracts over the partition
dim with no on-device transposes; softmax runs over the partition (key) dim
via a pairwise DVE/GPSIMD adder tree + gpsimd partition_all_reduce, and the
normalization is folded in after the attn@v matmul (divide by denom once on
[KH, T] instead of on [T, T]).

Matmul operand dtype is selectable (bf16 default; f32r = TF32-like; f32)
with fp32 PSUM accumulation throughout.
"""
import math
import os

import numpy as np

import concourse.bass as bass  # noqa: F401  (registers engines)
import concourse.mybir as mybir
from concourse import bacc
from concourse.bass_utils import run_bass_kernel_spmd
from concourse.tile import TileContext

F32 = mybir.dt.float32
AF = mybir.ActivationFunctionType

B, T, EMB, KH = 64, 1024, 128, 256
NCORES = 8
BPC = B // NCORES          # batches per core
ST = T // 128              # 8 key tiles of 128
TB = T // 512              # 2 moving-dim blocks of 512

DT_MM_NAME = os.environ.get("BASS_KERNEL_DT", "bf16")
_DT_MAP = {"bf16": mybir.dt.bfloat16, "f32r": mybir.dt.float32r, "f32": F32}

_CACHE = {}


def _build_nc(dt_mm):
    nc = bacc.Bacc("TRN2", target_bir_lowering=False, debug=False)

    xT = nc.declare_dram_parameter("xT", [BPC, EMB, T], dt_mm, isOutput=False)
    wkn = nc.declare_dram_parameter("wkN", [KH, EMB], dt_mm, isOutput=False)
    wqn = nc.declare_dram_parameter("wqN", [KH, EMB], dt_mm, isOutput=False)
    wv = nc.declare_dram_parameter("wvT", [EMB, KH], dt_mm, isOutput=False)
    pkw = nc.declare_dram_parameter("pkwT", [BPC, KH, KH], dt_mm, isOutput=False)
    pqw = nc.declare_dram_parameter("pqwT", [BPC, KH, KH], dt_mm, isOutput=False)
    # aux = [pkb | pqb | mb] packed: [BPC, 2*KH + T]
    aux = nc.declare_dram_parameter("aux", [BPC, 2 * KH + T], F32, isOutput=False)
    out = nc.declare_dram_parameter("out", [BPC, T, KH], F32, isOutput=True)
    KH1 = KH + 1  # v gets a ones column: attn @ [v | 1] yields the softmax denom

    with TileContext(nc) as tc:
        with tc.tile_pool(name="const", bufs=1) as cpool, \
             tc.tile_pool(name="work", bufs=3) as wpool, \
             tc.tile_pool(name="big", bufs=1) as bpool, \
             tc.tile_pool(name="psa", bufs=2, space="PSUM") as psa, \
             tc.tile_pool(name="psd", bufs=2, space="PSUM") as psdp, \
             tc.tile_pool(name="psx", bufs=2, space="PSUM") as psxp:

            # Wk/Wq natural [h, e] as 2 h-tiles side by side; WvT [e, d]
            wknt = cpool.tile([128, 2 * EMB], dt_mm, name="wknt")
            wqnt = cpool.tile([128, 2 * EMB], dt_mm, name="wqnt")
            wvt = cpool.tile([128, KH], dt_mm, name="wvt")
            nc.sync.dma_start(out=wknt.rearrange("p (hh e) -> p hh e", hh=2),
                              in_=wkn.ap().rearrange("(hh p) e -> p hh e", p=128))
            nc.sync.dma_start(out=wqnt.rearrange("p (hh e) -> p hh e", hh=2),
                              in_=wqn.ap().rearrange("(hh p) e -> p hh e", p=128))
            nc.sync.dma_start(out=wvt, in_=wv[:, :])

            # ---- fold person matrices into projection weights (all batches
            # upfront, so the steady-state loop never waits on this chain) ---
            # weffT[e, o] = sum_h W[h, e] * PW_T[h, o]
            wkeffs = [cpool.tile([128, KH], dt_mm, name=f"wkeff{b}") for b in range(BPC)]
            wqeffs = [cpool.tile([128, KH], dt_mm, name=f"wqeff{b}") for b in range(BPC)]
            with tc.tile_pool(name="pw", bufs=16) as pwpool:
                pwts = []
                for b in range(BPC):
                    for (j, pw_d) in ((0, pkw), (1, pqw)):
                        pwt = pwpool.tile([128, 2 * KH], dt_mm, name=f"pwt{b}_{j}", tag="pwt")
                        eng = nc.sync if (2 * b + j) % 2 == 0 else nc.gpsimd
                        eng.dma_start(
                            out=pwt.rearrange("p (hh o) -> p hh o", hh=2),
                            in_=pw_d[b].rearrange("(hh p) o -> p hh o", p=128))
                        pwts.append(pwt)
                for b in range(BPC):
                    for (j, wn, weff) in ((0, wknt, wkeffs[b]), (1, wqnt, wqeffs[b])):
                        pwt = pwts[2 * b + j]
                        pse = psa.tile([128, KH], F32, name="pse", tag="a")
                        for hh in range(2):
                            nc.tensor.matmul(pse, wn[:, hh * EMB:(hh + 1) * EMB],
                                             pwt[:, hh * KH:(hh + 1) * KH],
                                             start=(hh == 0), stop=(hh == 1))
                        nc.scalar.copy(weff, pse)

            for b in range(BPC):
                # ---- load per-batch operands -------------------------------
                xt = wpool.tile([128, T], dt_mm, name="xt")
                nc.sync.dma_start(out=xt, in_=xT[b])
                auxt = wpool.tile([128, 4 + ST], F32, name="auxt")
                nc.sync.dma_start(out=auxt, in_=aux[b].rearrange("(a p) -> p a", p=128))
                pkbt = auxt[:, 0:2]
                pqbt = auxt[:, 2:4]
                mbt = auxt[:, 4:4 + ST]

                # ---- projections ------------------------------------------
                # keysT/queriesT: [o, s] as [128, oh*T + s]
                # v (with ones column): [s, d] as [128, st*KH1 + d], col KH = 1.0
                kt = wpool.tile([128, 2 * T], dt_mm, name="kt")
                qt = wpool.tile([128, 2 * T], dt_mm, name="qt")
                vt = wpool.tile([128, ST * KH1], dt_mm, name="vt")
                for (weff, bt_, dst) in ((wkeffs[b], pkbt, kt), (wqeffs[b], pqbt, qt)):
                    for oh in range(2):
                        for sb in range(TB):
                            psp = psa.tile([128, 512], F32, name="psp", tag="a")
                            nc.tensor.matmul(psp, weff[:, oh * 128:(oh + 1) * 128],
                                             xt[:, sb * 512:(sb + 1) * 512])
                            nc.vector.tensor_scalar_add(
                                dst[:, oh * T + sb * 512:oh * T + (sb + 1) * 512],
                                psp, bt_[:, oh:oh + 1])
                nc.vector.memset(
                    vt.rearrange("p (st c) -> p st c", c=KH1)[:, :, KH:KH1], 1.0)
                for st in range(ST):
                    psv = psa.tile([128, KH], F32, name="psv", tag="a")
                    nc.tensor.matmul(psv, xt[:, st * 128:(st + 1) * 128], wvt)
                    nc.scalar.copy(vt[:, st * KH1:st * KH1 + KH], psv)

                # ---- attention scores + exp -------------------------------
                # dotT: [s, t]; E[s, t] = exp(dot + maskbias[s])
                et = bpool.tile([128, ST * T], dt_mm, name="et", bufs=1)
                for st in range(ST):
                    psd = psdp.tile([128, T], F32, name="psd", tag="d")
                    for dh in range(2):
                        lhs = kt[:, dh * T + st * 128:dh * T + (st + 1) * 128]
                        for tb in range(TB):
                            nc.tensor.matmul(
                                psd[:, tb * 512:(tb + 1) * 512], lhs,
                                qt[:, dh * T + tb * 512:dh * T + (tb + 1) * 512],
                                start=(dh == 0), stop=(dh == 1))
                    nc.scalar.activation(et[:, st * T:(st + 1) * T], psd,
                                         AF.Exp, bias=mbt[:, st:st + 1])

                # ---- context + softmax denominator in one matmul ----------
                # ctx_aug[t, 0:KH]  = sum_s E[s, t] * v[s, d]
                # ctx_aug[t, KH]    = sum_s E[s, t]          (the ones column)
                # Output lands in natural [t, d] layout; normalization is a
                # per-partition reciprocal+scale, no transpose anywhere.
                for tt in range(ST):
                    psx = psxp.tile([128, KH1], F32, name="psx", tag="x")
                    for st in range(ST):
                        nc.tensor.matmul(
                            psx, et[:, st * T + tt * 128:st * T + (tt + 1) * 128],
                            vt[:, st * KH1:(st + 1) * KH1],
                            start=(st == 0), stop=(st == ST - 1))
                    rcpc = wpool.tile([128, 1], F32, name="rcpc")
                    nc.vector.reciprocal(rcpc, psx[:, KH:KH1])
                    ctxn = wpool.tile([128, KH], F32, name="ctxn")
                    nc.vector.tensor_scalar_mul(ctxn, psx[:, 0:KH], rcpc)
                    nc.sync.dma_start(out=out[b, tt * 128:(tt + 1) * 128, :],
                                      in_=ctxn)

    nc.compile()
    return nc


def _get_nc():
    if "nc" not in _CACHE:
        _CACHE["nc"] = _build_nc(_DT_MAP[DT_MM_NAME])
    return _CACHE["nc"]


def _np_mm_dtype():
    if DT_MM_NAME == "bf16":
        import ml_dtypes
        return np.dtype(ml_dtypes.bfloat16)
    return np.float32


def build_in_maps(x, mask, person_idxs, Wk, Wq, Wv, PK_W, PK_b, PQ_W, PQ_b):
    x = np.asarray(x, dtype=np.float32)
    mask = np.asarray(mask)
    idx = np.asarray(person_idxs).astype(np.int64)
    sk = 1.0 / math.sqrt(KH)
    mdt = _np_mm_dtype()

    wkN = np.ascontiguousarray(np.asarray(Wk, np.float32)).astype(mdt)   # [KH, EMB]
    wqN = np.ascontiguousarray(np.asarray(Wq, np.float32)).astype(mdt)
    wvT = np.ascontiguousarray(np.asarray(Wv, np.float32).T).astype(mdt)
    mbias = np.where(mask[:, 0, :], 0.0, -30.0).astype(np.float32)  # [B, T]

    in_maps = []
    for c in range(NCORES):
        bs = slice(c * BPC, (c + 1) * BPC)
        ci = idx[bs]
        in_maps.append({
            "xT": np.ascontiguousarray(x[bs].transpose(0, 2, 1)).astype(mdt),
            "wkN": wkN, "wqN": wqN, "wvT": wvT,
            "pkwT": np.ascontiguousarray(
                np.asarray(PK_W, np.float32)[ci].transpose(0, 2, 1)).astype(mdt),
            "pqwT": np.ascontiguousarray(
                (np.asarray(PQ_W, np.float32)[ci] * sk).transpose(0, 2, 1)).astype(mdt),
            "aux": np.ascontiguousarray(np.concatenate([
                np.asarray(PK_b, np.float32)[ci],
                np.asarray(PQ_b, np.float32)[ci] * sk,
                mbias[bs]], axis=1)),
        })
    return in_maps


def kernel(x, mask, person_idxs, Wk, Wq, Wv, PK_W, PK_b, PQ_W, PQ_b):
    in_maps = build_in_maps(x, mask, person_idxs, Wk, Wq, Wv, PK_W, PK_b, PQ_W, PQ_b)
    nc = _get_nc()
    res = run_bass_kernel_spmd(nc, in_maps, list(range(NCORES)))
    return np.concatenate([res.results[c]["out"] for c in range(NCORES)], axis=0)
